# revision 6
# baseline (speedup 1.0000x reference)
"""Single-head causal attention with tanh soft-capping on 8 TRN2 NeuronCores.

Problem: nn_Attention_30056181138106
  input [8, 2048, 1024] f32, attention_mask [8, 2048] i32 (ones),
  W_Q/W_K/W_V [128, 1024] f32.
  out[b] = softmax(causal_mask(30*tanh((x Wq^T)(x Wk^T)^T / sqrt(128)))) @ (x Wv^T)

Sharding: data-parallel over batch, one batch element per core, weights
replicated. No collectives needed.

Per-core algorithm (transposed-score formulation), q-chunks of 256 columns,
software-pipelined so attention for chunk c overlaps prep for chunk c+1:
  xT[dm, L]  = PE-transpose of x (f32r data+identity -> 1.5 c/col),
               PSUM evacuated by GPSIMD (Pool) tensor_copy to keep DVE free
  QT, KT     = Wq/Wk-proj as [dh, L] f32r (PSUM fp32 accumulate over 8 dm tiles)
  Vn         = Wv-proj transposed back to natural [L, dh] f32r
  ST group   = up-to-4 k-tiles of scores -> PSUM [k=128, ab, q=256], f32r,
               full tiles (no causal trimming; tanh caps any value so the
               garbage region is finite and masked later)
  t          = tanh(ST / sqrt(dh))        (one ScalarE call per group)
  E          = exp(30*t - 30 + mask_bias) (one ScalarE call per group -> f32r;
               weights in (0,1], shift cancels in normalization)
  diag tiles: one DVE multiply with a 0/1 causal mask zeroes the upper wedge
               (and the fully-masked prefix), so downstream is uniform
  O_unnorm   = sum_k Vn_kt^T E            (PSUM fp32 accumulate [dh, 256])
  den^T      = per 128-col subtile: matmul(lhsT=E_sub[128k,128q], rhs=ones)
               -> PSUM [q=128, 1] accumulated over k-tiles: den lands in
               natural q-partition layout at ~1 cycle per matmul
  normalize: recip(den^T) on DVE, PE-transpose O to natural layout, one
  per-partition tensor_scalar multiply per 128-row tile, DMA out.

Softmax max-subtraction is unnecessary: scores are capped to [-30, 30] by
tanh, so exp(s-30) is in (0, 1] and all sums stay in fp32 range. float32r
matmuls measure ~1.6e-4 max matmul error on TRN2 (better than fp16/bf16)
at full PE rate for moving dim >= 256.

The batched-ACT fast path assumes attention_mask is all ones (bias is the
constant -30); kernel() checks the mask at run time and falls back to a
per-k-tile-bias variant when any key is masked.
"""

import numpy as np
from math import sqrt
from contextlib import ExitStack

import concourse.bass as bass
import concourse.mybir as mybir
from concourse import bacc
from concourse.tile import TileContext
from concourse.bass_utils import run_bass_kernel_spmd
from concourse.masks import make_identity

B, L, DM, DH = 8, 2048, 1024, 128
TAU = 30.0
ISQ = 1.0 / sqrt(DH)
NEG_BIAS = -10000.0  # exp(x + NEG_BIAS) == 0.0 for any capped x

F32 = mybir.dt.float32
F32R = mybir.dt.float32r
BF16 = mybir.dt.bfloat16
I32 = mybir.dt.int32
AF = mybir.ActivationFunctionType

QC = 256            # q-chunk width
NQC = L // QC       # 8 q-chunks
TPC = QC // 128     # 2 q-tiles per chunk
NLT = L // 128      # 16 l-tiles
NDT = DM // 128     # 8 dm-tiles

_CACHE = {}
_BUILD_OPTS = {}


def _build_nc(mask_ones: bool, ab: int = 4, mm_bufs: int = 2, tr_bufs: int = 2,
              xs_bufs: int = 3, work_bufs: int = 2, outp_bufs: int = 2,
              pool_xt: bool = True):
    # Bacc (not bare Bass): its finalize() runs move_matmul_waits_to_ldweights
    # + generate_event_semaphores, required by walrus wait-count limits.
    nc = bacc.Bacc(None, target_bir_lowering=False)
    # f32r relabel of the f32 inputs: same bits, full-rate PE matmuls.
    x = nc.declare_dram_parameter("x", [L, DM], F32R, isOutput=False)
    am = nc.declare_dram_parameter("attention_mask", [L], I32, isOutput=False)
    wq = nc.declare_dram_parameter("W_Q", [DH, DM], F32R, isOutput=False)
    wk = nc.declare_dram_parameter("W_K", [DH, DM], F32R, isOutput=False)
    wv = nc.declare_dram_parameter("W_V", [DH, DM], F32R, isOutput=False)
    out = nc.declare_dram_parameter("out", [L, DH], F32, isOutput=True)

    with TileContext(nc) as tc:
        with ExitStack() as ctx:
            sb = ctx.enter_context(tc.tile_pool(name="sb", bufs=1))
            stage = ctx.enter_context(tc.tile_pool(name="stage", bufs=xs_bufs))
            work = ctx.enter_context(tc.tile_pool(name="work", bufs=work_bufs))
            outp = ctx.enter_context(tc.tile_pool(name="outp", bufs=outp_bufs))
            # PSUM: tr (transposes + proj) + mm (score groups) + acc (po|pd)
            pp_tr = ctx.enter_context(tc.tile_pool(name="pp_tr", bufs=tr_bufs, space="PSUM"))
            pp_mm = ctx.enter_context(tc.tile_pool(name="pp_mm", bufs=mm_bufs, space="PSUM"))
            pp_acc = ctx.enter_context(tc.tile_pool(name="pp_acc", bufs=1, space="PSUM"))
            pp_den = ctx.enter_context(tc.tile_pool(name="pp_den", bufs=1, space="PSUM"))

            # --- constants ---
            # f32r identity: the identity is the moving operand of a PE
            # transpose, so its dtype sets the stream rate (1.5 cyc/col vs
            # 2.0 for plain f32); walrus requires both matmul operands to be
            # the same 32-bit dtype, and transposes are exact permutations.
            ident = sb.tile([128, 128], F32R, name="ident")
            make_identity(nc, ident)
            ones_fr = sb.tile([128, 1], F32, name="ones_f")
            nc.vector.memset(ones_fr, 1.0)
            ones = sb.tile([128, 1], F32R, name="ones")
            nc.vector.tensor_copy(ones, ones_fr)
            bias_m30 = sb.tile([128, 1], F32, name="bias_m30")
            nc.vector.memset(bias_m30, -TAU)

            # 0/1 causal masks for the 2 diagonal offsets of a 256-wide
            # chunk: keep q - 128*di - p >= 0 (zeroes the wedge AND the
            # fully-masked q < 128*di prefix).
            cmasks = sb.tile([128, TPC, QC], BF16, name="cmasks")
            for i in range(TPC):
                nc.vector.memset(cmasks[:, i, :], 1.0)
                nc.gpsimd.affine_select(
                    out=cmasks[:, i, :], in_=cmasks[:, i, :],
                    compare_op=mybir.AluOpType.is_ge, fill=0.0,
                    base=-128 * i, channel_multiplier=-1, pattern=[[1, QC]],
                )

            mbias = None
            if not mask_ones:
                # key-padding mask -> additive exp bias: m*10000 - 10030
                am_i = sb.tile([128, NLT], I32, name="am_i")
                nc.sync.dma_start(out=am_i, in_=am[:].rearrange("(t p) -> p t", p=128))
                am_f = sb.tile([128, NLT], F32, name="am_f")
                nc.vector.tensor_copy(am_f, am_i)
                mbias = sb.tile([128, NLT], F32, name="mbias")
                nc.vector.tensor_scalar(
                    out=mbias, in0=am_f, scalar1=-NEG_BIAS, scalar2=NEG_BIAS - TAU,
                    op0=mybir.AluOpType.mult, op1=mybir.AluOpType.add,
                )

            # --- weights: load + PE-transpose (batched copies) ---
            wTs = {}
            for nm, wh in (("q", wq), ("k", wk), ("v", wv)):
                ws = stage.tile([128, DM], F32R, name=f"ws_{nm}", tag="ws")
                nc.sync.dma_start(out=ws, in_=wh[:, :])
                wT = sb.tile([128, NDT, 128], F32R, name=f"wT_{nm}")
                for g in range(2):
                    ps = pp_tr.tile([128, 512], F32R, name=f"ps_w{nm}{g}", tag="tr")
                    for i in range(4):
                        dt = g * 4 + i
                        nc.tensor.transpose(
                            ps[:, i * 128:(i + 1) * 128],
                            ws[:, dt * 128:(dt + 1) * 128], ident)
                    nc.vector.tensor_copy(
                        wT[:, g * 4:(g + 1) * 4, :],
                        ps.rearrange("p (a b) -> p a b", a=4))
                wTs[nm] = wT

            xT = sb.tile([128, NDT, L], F32R, name="xT")
            QT = sb.tile([128, L], F32R, name="QT")
            KT = sb.tile([128, L], F32R, name="KT")
            Vn = sb.tile([128, L], F32R, name="Vn")

            def prep_pieces(c):
                """Emit-later closures for chunk c's load/transpose/proj."""
                cs = slice(c * QC, (c + 1) * QC)
                box = {}

                def dma():
                    xs = stage.tile([128, TPC, DM], F32R, name="xs", tag="xs")
                    nc.sync.dma_start(
                        out=xs, in_=x[cs, :].rearrange("(j p) d -> p j d", p=128))
                    box["xs"] = xs

                def tr(j, g):
                    def go():
                        lt = TPC * c + j
                        ps = pp_tr.tile([128, 512], F32R, name="ps_tr", tag="tr")
                        for i in range(4):
                            dt = g * 4 + i
                            nc.tensor.transpose(
                                ps[:, i * 128:(i + 1) * 128],
                                box["xs"][:, j, dt * 128:(dt + 1) * 128], ident)
                        dst = xT[:, g * 4:(g + 1) * 4, lt * 128:(lt + 1) * 128]
                        src = ps.rearrange("p (a b) -> p a b", a=4)
                        if pool_xt:
                            nc.gpsimd.tensor_copy(dst, src)
                        else:
                            nc.vector.tensor_copy(dst, src)
                    return go

                def proj(nm):
                    def go():
                        pm = pp_tr.tile([128, QC], F32, name=f"pm_{nm}", tag="tr")
                        for dt in range(NDT):
                            nc.tensor.matmul(
                                pm, lhsT=wTs[nm][:, dt, :], rhs=xT[:, dt, cs],
                                start=(dt == 0), stop=(dt == NDT - 1),
                            )
                        if nm == "v":
                            vt_c = work.tile([128, QC], F32R, name="vt_c")
                            nc.vector.tensor_copy(vt_c, pm)
                            ps = pp_tr.tile([128, QC], F32R, name="ps_vn", tag="tr")
                            for j in range(TPC):
                                nc.tensor.transpose(
                                    ps[:, j * 128:(j + 1) * 128],
                                    vt_c[:, j * 128:(j + 1) * 128], ident)
                            nc.vector.tensor_copy(Vn[:, cs], ps)
                        else:
                            nc.vector.tensor_copy(
                                (QT if nm == "q" else KT)[:, cs], pm)
                    return go

                pieces = [tr(j, g) for j in range(TPC) for g in range(2)]
                pieces += [proj("q"), proj("k"), proj("v")]
                return dma, pieces

            def attn(c, inject):
                """Attention for q-chunk c; inject = prep closures for c+1,
                interleaved between score groups to keep the PE fed while
                ScalarE runs tanh/exp."""
                cs = slice(c * QC, (c + 1) * QC)
                nkt = TPC * c + TPC
                ngr = -(-nkt // ab)
                po = pp_acc.tile([128, QC], F32, name="po")
                pd = pp_den.tile([128, TPC], F32, name="pd")
                inj = list(inject)
                per_g = -(-len(inj) // ngr) if inj else 0

                def scores(g):
                    k0 = g * ab
                    nt = min(ab, nkt - k0)
                    pbig = pp_mm.tile([128, ab, QC], F32, name="pbig", tag="mm")
                    for i in range(nt):
                        kt = k0 + i
                        nc.tensor.matmul(
                            pbig[:, i, :], lhsT=KT[:, kt * 128:(kt + 1) * 128],
                            rhs=QT[:, cs], start=True, stop=True,
                        )
                    return pbig, nt

                pbig, nt = scores(0)
                for g in range(ngr):
                    k0 = g * ab
                    t_big = work.tile([128, ab, QC], F32, name="t_big")
                    e_big = work.tile([128, ab, QC], F32R, name="e_big")
                    nc.scalar.activation(
                        t_big[:, :nt, :], pbig[:, :nt, :], AF.Tanh, scale=ISQ)
                    if mask_ones:
                        nc.scalar.activation(
                            e_big[:, :nt, :], t_big[:, :nt, :], AF.Exp,
                            bias=bias_m30, scale=TAU)
                    else:
                        for i in range(nt):
                            kt = k0 + i
                            nc.scalar.activation(
                                e_big[:, i, :], t_big[:, i, :], AF.Exp,
                                bias=mbias[:, kt:kt + 1], scale=TAU)
                    # keep the PE busy during tanh/exp: next score group,
                    # then a slice of chunk c+1's prep
                    if g + 1 < ngr:
                        pbig2, nt2 = scores(g + 1)
                    for p in inj[g * per_g:(g + 1) * per_g]:
                        p()
                    # diagonal tiles: zero the causal wedge + masked prefix
                    for i in range(nt):
                        di = k0 + i - TPC * c
                        if di >= 0:
                            w = 128 * (di + 1)
                            nc.vector.tensor_mul(
                                e_big[:, i, :w], e_big[:, i, :w],
                                cmasks[:, di, :w])
                    for i in range(nt):
                        kt = k0 + i
                        nc.tensor.matmul(
                            po, lhsT=Vn[:, kt * 128:(kt + 1) * 128],
                            rhs=e_big[:, i, :],
                            start=(kt == 0), stop=(kt == nkt - 1),
                        )
                        for sub in range(TPC):
                            nc.tensor.matmul(
                                pd[:, sub:sub + 1],
                                lhsT=e_big[:, i, sub * 128:(sub + 1) * 128],
                                rhs=ones,
                                start=(kt == 0 and sub == 0),
                                stop=(kt == nkt - 1 and sub == TPC - 1),
                            )
                    if g + 1 < ngr:
                        pbig, nt = pbig2, nt2
                for p in inj[ngr * per_g:]:
                    p()

                # normalize in natural layout: den^T is already q-major
                rden = work.tile([128, TPC], F32, name="rden")
                nc.vector.reciprocal(rden, pd)
                on_sb = work.tile([128, QC], F32R, name="on_sb")
                nc.vector.tensor_copy(on_sb, po)
                ps_o = pp_tr.tile([128, QC], F32R, name="ps_o", tag="tr")
                for j in range(TPC):
                    nc.tensor.transpose(
                        ps_o[:, j * 128:(j + 1) * 128],
                        on_sb[:, j * 128:(j + 1) * 128], ident)
                o_sb = outp.tile([128, TPC, 128], F32, name="o_sb")
                for j in range(TPC):
                    nc.vector.tensor_scalar_mul(
                        o_sb[:, j, :], ps_o[:, j * 128:(j + 1) * 128],
                        rden[:, j:j + 1])
                nc.sync.dma_start(
                    out=out[cs, :].rearrange("(j p) d -> p j d", p=128),
                    in_=o_sb)

            dma0, pieces0 = prep_pieces(0)
            dma0()
            dma1, pieces1 = prep_pieces(1)
            dma1()
            for p in pieces0:
                p()
            nxt = pieces1
            for c in range(NQC):
                if c + 2 < NQC:
                    dma_n, pieces_n = prep_pieces(c + 2)
                    dma_n()
                else:
                    pieces_n = []
                attn(c, inject=nxt)
                nxt = pieces_n
    if not nc.is_finalized():
        nc.finalize()
    return nc


def _get_nc(mask_ones: bool):
    key = ("nc", mask_ones)
    if key not in _CACHE:
        _CACHE[key] = _build_nc(mask_ones, **_BUILD_OPTS)
    return _CACHE[key]


def kernel(**inputs) -> np.ndarray:
    x = np.ascontiguousarray(np.asarray(inputs["input"], dtype=np.float32))
    am = np.ascontiguousarray(np.asarray(inputs["attention_mask"], dtype=np.int32))
    wq = np.ascontiguousarray(np.asarray(inputs["W_Q"], dtype=np.float32))
    wk = np.ascontiguousarray(np.asarray(inputs["W_K"], dtype=np.float32))
    wv = np.ascontiguousarray(np.asarray(inputs["W_V"], dtype=np.float32))

    nc = _get_nc(bool((am == 1).all()))
    in_maps = [
        {"x": x[b], "attention_mask": am[b], "W_Q": wq, "W_K": wk, "W_V": wv}
        for b in range(B)
    ]
    res = run_bass_kernel_spmd(nc, in_maps, list(range(B))).results
    return np.stack([res[b]["out"] for b in range(B)]).astype(np.float32)
